# revision 27
# baseline (speedup 1.0000x reference)
"""Trainium2 Bass kernel for nn_CAGKE_1 (Gaussian-kernel embedding).

Math: reference computes, for mask m_i = 1[X_i > 0.5],
    out[j] = sum_e softmax(w)_e * sum_i m_i * (c/sigma_e) exp(-(j-i-1)^2/(2 sigma_e^2)) + noise_j
Both sums are linear, so the E=128 Gaussian channels collapse into one
combined kernel ghat(d) = sum_e softmax(w)_e * (c/sigma_e) exp(-d^2/(2 sigma_e^2))
BEFORE the convolution. With sigma in [0.5, 5], a 32-tap kernel (|d| <= 16)
captures all but ~1e-4 of the mass (gate is 2e-2), and ghat is even, so the
tap order never needs reversing.

Per core (1024 outputs, no collectives, no DRAM round trips):
  - The host stages the MASK Toeplitz operand directly: xstack[p, j] =
    Xpad[1024c + 256*(p//32) + (p%32) + j], i.e. 4 output quarters stacked
    32-tap-deep on the partition axis, with w^T and sigma as trailing
    columns; split into two [64, 258] DMAs so both HWDGE rings run.
  - Device binarizes to bf16 (one DVE op). The exp table fuses the softmax
    numerator into the ACT bias: expt_w[e, p] = exp(-d_p^2/(2 s_e^2) + w_e),
    with -d^2 built negated via two iotas and 2*sigma^2 via the ACT Square
    (same table set as Exp), so only one DVE reciprocal gates the table.
    ghat_col = expt_w^T @ (1/sigma) in one matmul; the softmax denominator
    Z = sum_e exp(w_e) comes from a second matmul against a preloaded
    all-ones weight (no transposes, no reduces).
  - Conv = 4 CONCURRENT 32x32-tile matmuls (tile_position=(32q, 32q)), each
    out[1, 256] = ghat_col[32q:+32]^T @ xstack-quarter, landing on psum
    partitions {0, 32, 64, 96} of one bank.
  - Noise is host-packed to the same sparse partitions; one DVE add fuses
    PSUM evacuation + noise; a 4-descriptor DMA stores [4, 256].
"""

import sys

import numpy as np

if "/opt/trn_rl_repo" not in sys.path:
    sys.path.insert(0, "/opt/trn_rl_repo")

T = 8192
E = 128
N_CORES = 8
TJ = T // N_CORES          # 1024 outputs per core
NQ = 4                     # output quarters per core
QW = TJ // NQ              # 256 outputs per quarter
KT = 32                    # taps: d in [-15, 16] (even kernel, order-free)
PADL = 17                  # = KT//2 + 1: mask idx j-17+k, k in [0,32)
PADR = 16
INV_SQRT_2PI = 0.39894228

_compiled = None


def _build():
    import concourse.bacc as bacc
    import concourse.bass as bass
    import concourse.mybir as mybir
    import concourse.tile as tile

    f32 = mybir.dt.float32
    bf16 = mybir.dt.bfloat16
    nc = bacc.Bacc(num_devices=N_CORES, debug=False)

    xsa_d = nc.dram_tensor("xstacka", [64, QW + 2], f32, kind="ExternalInput")
    xsb_d = nc.dram_tensor("xstackb", [64, QW + 2], f32, kind="ExternalInput")
    nw_d = nc.dram_tensor("noisew", [NQ, QW], f32, kind="ExternalInput")
    out_d = nc.dram_tensor("out", [NQ, QW], f32, kind="ExternalOutput")

    with tile.TileContext(nc) as tc:
        with (
            tc.tile_pool(name="pool", bufs=1) as pool,
            tc.tile_pool(name="psum", bufs=1, space="PSUM") as psum,
        ):
            # ---- input loads: mask stack halves on both HWDGE rings ----
            xs = pool.tile([128, QW + 2], f32, tag="xs")
            nc.sync.dma_start(xs[0:64, :], xsa_d[:])
            nc.scalar.dma_start(xs[64:128, :], xsb_d[:])
            wT = xs[:, QW : QW + 1]
            sgT = xs[:, QW + 1 : QW + 2]

            nw = pool.tile([128, QW], f32, tag="nw")
            nw_ap = bass.AP(nw[:].tensor, 0, [[32 * QW, NQ], [1, QW]])
            nc.sync.dma_start(nw_ap, nw_d[:])

            # dummy Exp forces the ACT exp-table load off the critical path
            dum = pool.tile([1, 1], f32, tag="dum")
            nc.vector.memset(dum[:], 0.0)
            nc.scalar.activation(dum[:], dum[:], mybir.ActivationFunctionType.Exp)
            ones = pool.tile([128, 128], bf16, tag="ones")
            nc.gpsimd.memset(ones[:], 1.0)

            # ---- input-independent prep: d2[e, p] = -((p % 32) - 16)^2
            # built NEGATED via two iotas so no negate op is ever needed ----
            d2i = pool.tile([128, 128], f32, tag="d2i")
            nc.gpsimd.iota(
                d2i[:], pattern=[[0, NQ], [1, KT]], base=-(KT // 2),
                channel_multiplier=0, allow_small_or_imprecise_dtypes=True,
            )
            d2n = pool.tile([128, 128], f32, tag="d2n")
            nc.gpsimd.iota(
                d2n[:], pattern=[[0, NQ], [-1, KT]], base=(KT // 2),
                channel_multiplier=0, allow_small_or_imprecise_dtypes=True,
            )
            d2 = pool.tile([128, 128], f32, tag="d2")
            nc.vector.tensor_mul(d2[:], d2i[:], d2n[:])

            # ---- sigma chain: 2 sigma^2 on ACT (same table set as Exp),
            # one DVE reciprocal -> +1/(2 sigma^2); exp(w) for Z ----
            sq2 = pool.tile([128, 1], f32, tag="sq2")
            nc.scalar.activation(
                sq2[:], sgT, mybir.ActivationFunctionType.Square,
                scale=1.4142135623730951,
            )
            exc = pool.tile([128, 1], bf16, tag="exc")
            nc.scalar.activation(
                exc[:], wT, mybir.ActivationFunctionType.Exp
            )
            invs = pool.tile([128, 1], f32, tag="invs")
            nc.vector.reciprocal(invs[:], sq2[:])
            rs = pool.tile([128, 1], f32, tag="rs")
            nc.vector.reciprocal(rs[:], sgT)
            rsb = pool.tile([128, 1], bf16, tag="rsb")
            nc.vector.tensor_scalar_mul(rsb[:], rs[:], 1.0)

            # ---- Z = sum_e exp(w_e) on all partitions via preloaded ones
            zp = psum.tile([128, 1], f32, tag="zp")
            nc.tensor.matmul(zp[:], ones[:], exc[:], start=True, stop=True)
            rz = pool.tile([128, 1], f32, tag="rz")
            nc.vector.reciprocal(rz[:], zp[:])

            # ---- binarize mask Toeplitz -> bf16 ----
            mT = pool.tile([128, QW], bf16, tag="mT")
            nc.vector.tensor_scalar(
                mT[:], xs[:, 0:QW], 0.5, None, mybir.AluOpType.is_gt
            )

            # ---- exp table with softmax numerator fused into the bias:
            # expt_w[e, p] = exp(-d_p^2/(2 s_e^2) + w_e) = exp(w_e) G_e(d_p)
            expt = pool.tile([128, 128], bf16, tag="expt")
            nc.scalar.activation(
                expt[:], d2[:], mybir.ActivationFunctionType.Exp,
                bias=wT, scale=invs[:],
            )

            # ---- ghat_col[p] = (c/Z) sum_e expt_w[e, p] / sigma_e ----
            ghat_p = psum.tile([128, 1], f32, tag="ghat_p")
            nc.tensor.matmul(ghat_p[:], expt[:], rsb[:], start=True, stop=True)
            ghat = pool.tile([128, 1], bf16, tag="ghat")
            nc.vector.tensor_scalar(
                ghat[:], ghat_p[:], rz[:], INV_SQRT_2PI,
                mybir.AluOpType.mult, mybir.AluOpType.mult,
            )

            # ---- conv: 4 concurrent 32x32-tile matmuls, quarter q ->
            # psum partition 32q: out[1, 256] = ghat[32q:+32]^T @ mT[32q:+32, :]
            op = psum.tile([128, QW], f32, tag="op")
            for q in range(NQ):
                nc.tensor.matmul(
                    op[32 * q : 32 * q + 1, :],
                    ghat[32 * q : 32 * q + 32, :],
                    mT[32 * q : 32 * q + 32, :],
                    start=True, stop=True,
                    tile_position=(32 * q, 32 * q),
                )

            # ---- fused PSUM evacuation + noise add; sparse-partition store
            outS = pool.tile([128, QW], f32, tag="outS")
            nc.vector.tensor_add(outS[:], op[:], nw[:])
            outS_ap = bass.AP(outS[:].tensor, 0, [[32 * QW, NQ], [1, QW]])
            nc.sync.dma_start(out_d[:], outS_ap)

    nc.compile()
    return nc


def kernel(X, sigma, weight, noise):
    global _compiled
    from concourse.bass_utils import run_bass_kernel_spmd

    X = np.ascontiguousarray(np.asarray(X, dtype=np.float32)).reshape(1, T)
    sigma = np.ascontiguousarray(np.asarray(sigma, dtype=np.float32)).reshape(E)
    weight = np.ascontiguousarray(np.asarray(weight, dtype=np.float32)).reshape(1, E)
    noise = np.ascontiguousarray(np.asarray(noise, dtype=np.float32)).reshape(1, T)

    if _compiled is None:
        _compiled = _build()
    nc = _compiled

    Xp = np.zeros(PADL + T + PADR, dtype=np.float32)
    Xp[PADL : PADL + T] = X[0]
    # windows[s] = Xp[s : s+QW]; row p of core c starts at 1024c + 256(p//32) + (p%32)
    win = np.lib.stride_tricks.sliding_window_view(Xp, QW)
    p = np.arange(128)
    row_off = QW * (p // KT) + (p % KT)

    in_maps = []
    for c in range(N_CORES):
        xsfull = np.empty((128, QW + 2), dtype=np.float32)
        xsfull[:, 0:QW] = win[c * TJ + row_off]
        xsfull[:, QW] = weight[0]
        xsfull[:, QW + 1] = sigma
        nw = np.ascontiguousarray(
            noise[0, c * TJ : (c + 1) * TJ].reshape(NQ, QW)
        )
        in_maps.append({
            "xstacka": np.ascontiguousarray(xsfull[0:64]),
            "xstackb": np.ascontiguousarray(xsfull[64:128]),
            "noisew": nw,
        })

    res = run_bass_kernel_spmd(nc, in_maps, core_ids=list(range(N_CORES)))
    out = np.empty((1, T), dtype=np.float32)
    for c in range(N_CORES):
        out[0, c * TJ : (c + 1) * TJ] = res.results[c]["out"].reshape(-1)
    return out


# revision 29
# speedup vs baseline: 1.0354x; 1.0354x over previous
"""Trainium2 Bass kernel for nn_CAGKE_1 (Gaussian-kernel embedding).

Math: reference computes, for mask m_i = 1[X_i > 0.5],
    out[j] = sum_e softmax(w)_e * sum_i m_i * (c/sigma_e) exp(-(j-i-1)^2/(2 sigma_e^2)) + noise_j
Both sums are linear, so the E=128 Gaussian channels collapse into one
combined kernel ghat(d) = sum_e softmax(w)_e * (c/sigma_e) exp(-d^2/(2 sigma_e^2))
BEFORE the convolution. With sigma in [0.5, 5], a 32-tap kernel (|d| <= 16)
captures all but ~1e-4 of the mass (gate is 2e-2), and ghat is even, so the
tap order never needs reversing.

Per core (1024 outputs, no collectives, no DRAM round trips):
  - The host stages the MASK Toeplitz operand directly: xstack[p, j] =
    Xpad[1024c + 256*(p//32) + (p%32) + j], i.e. 4 output quarters stacked
    32-tap-deep on the partition axis, with w^T and sigma as trailing
    columns; split into two [64, 258] DMAs so both HWDGE rings run.
  - Device binarizes to bf16 (one DVE op). The exp table fuses the softmax
    numerator into the ACT bias: expt_w[e, p] = exp(-d_p^2/(2 s_e^2) + w_e),
    with -d^2 built negated via two iotas and 2*sigma^2 via the ACT Square
    (same table set as Exp), so only one DVE reciprocal gates the table.
    ghat_col = expt_w^T @ (1/sigma) in one matmul; the softmax denominator
    Z = sum_e exp(w_e) comes from a second matmul against a preloaded
    all-ones weight (no transposes, no reduces).
  - Conv = 4 CONCURRENT 32x32-tile matmuls (tile_position=(32q, 32q)), each
    out[1, 256] = ghat_col[32q:+32]^T @ xstack-quarter, landing on psum
    partitions {0, 32, 64, 96} of one bank.
  - Noise is host-packed to the same sparse partitions; one DVE add fuses
    PSUM evacuation + noise; a 4-descriptor DMA stores [4, 256].
"""

import sys

import numpy as np

if "/opt/trn_rl_repo" not in sys.path:
    sys.path.insert(0, "/opt/trn_rl_repo")

T = 8192
E = 128
N_CORES = 8
TJ = T // N_CORES          # 1024 outputs per core
NQ = 4                     # output quarters per core
QW = TJ // NQ              # 256 outputs per quarter
KT = 32                    # taps: d in [-15, 16] (even kernel, order-free)
PADL = 17                  # = KT//2 + 1: mask idx j-17+k, k in [0,32)
PADR = 16
INV_SQRT_2PI = 0.39894228

_compiled = None


def _build():
    import concourse.bacc as bacc
    import concourse.bass as bass
    import concourse.mybir as mybir
    import concourse.tile as tile

    f32 = mybir.dt.float32
    bf16 = mybir.dt.bfloat16
    nc = bacc.Bacc(num_devices=N_CORES, debug=False)

    xsa_d = nc.dram_tensor("xstacka", [64, QW + 2], f32, kind="ExternalInput")
    xsb_d = nc.dram_tensor("xstackb", [64, QW + 2], f32, kind="ExternalInput")
    nw_d = nc.dram_tensor("noisew", [NQ, QW], f32, kind="ExternalInput")
    out_d = nc.dram_tensor("out", [NQ, QW], f32, kind="ExternalOutput")

    with tile.TileContext(nc) as tc:
        with (
            tc.tile_pool(name="pool", bufs=1) as pool,
            tc.tile_pool(name="psum", bufs=1, space="PSUM") as psum,
        ):
            # ---- input loads: mask stack halves on both HWDGE rings ----
            xs = pool.tile([128, QW + 2], f32, tag="xs")
            nc.sync.dma_start(xs[0:64, :], xsa_d[:])
            nc.scalar.dma_start(xs[64:128, :], xsb_d[:])
            wT = xs[:, QW : QW + 1]
            sgT = xs[:, QW + 1 : QW + 2]

            nw = pool.tile([128, QW], f32, tag="nw")
            nw_ap = bass.AP(nw[:].tensor, 0, [[32 * QW, NQ], [1, QW]])
            nc.sync.dma_start(nw_ap, nw_d[:])

            # dummy Exp forces the ACT exp-table load off the critical path
            dum = pool.tile([1, 1], f32, tag="dum")
            nc.vector.memset(dum[:], 0.0)
            nc.scalar.activation(dum[:], dum[:], mybir.ActivationFunctionType.Exp)
            ones = pool.tile([128, 128], bf16, tag="ones")
            nc.gpsimd.memset(ones[:], 1.0)

            # ---- input-independent prep: d2[e, k] = -(k - 16)^2, k in [0,32)
            # built NEGATED via two iotas so no negate op is ever needed ----
            d2i = pool.tile([128, KT], f32, tag="d2i")
            nc.gpsimd.iota(
                d2i[:], pattern=[[1, KT]], base=-(KT // 2),
                channel_multiplier=0, allow_small_or_imprecise_dtypes=True,
            )
            d2n = pool.tile([128, KT], f32, tag="d2n")
            nc.gpsimd.iota(
                d2n[:], pattern=[[-1, KT]], base=(KT // 2),
                channel_multiplier=0, allow_small_or_imprecise_dtypes=True,
            )
            d2 = pool.tile([128, KT], f32, tag="d2")
            nc.vector.tensor_mul(d2[:], d2i[:], d2n[:])

            # ---- sigma chain: 2 sigma^2 on ACT (same table set as Exp),
            # one DVE reciprocal -> +1/(2 sigma^2); exp(w) for Z ----
            sq2 = pool.tile([128, 1], f32, tag="sq2")
            nc.scalar.activation(
                sq2[:], sgT, mybir.ActivationFunctionType.Square,
                scale=1.4142135623730951,
            )
            exc = pool.tile([128, 1], bf16, tag="exc")
            nc.scalar.activation(
                exc[:], wT, mybir.ActivationFunctionType.Exp
            )
            invs = pool.tile([128, 1], f32, tag="invs")
            nc.vector.reciprocal(invs[:], sq2[:])
            rs = pool.tile([128, 1], f32, tag="rs")
            nc.vector.reciprocal(rs[:], sgT)
            rsb = pool.tile([128, 1], bf16, tag="rsb")
            nc.vector.tensor_scalar_mul(rsb[:], rs[:], 1.0)

            # ---- Z = sum_e exp(w_e) on all partitions via preloaded ones
            zp = psum.tile([128, 1], f32, tag="zp")
            nc.tensor.matmul(zp[:], ones[:], exc[:], start=True, stop=True)
            rz = pool.tile([128, 1], f32, tag="rz")
            nc.vector.reciprocal(rz[:], zp[:])

            # ---- binarize mask Toeplitz -> bf16 ----
            mT = pool.tile([128, QW], bf16, tag="mT")
            nc.vector.tensor_scalar(
                mT[:], xs[:, 0:QW], 0.5, None, mybir.AluOpType.is_gt
            )

            # ---- exp table with softmax numerator fused into the bias:
            # expt_w[e, k] = exp(-d_k^2/(2 s_e^2) + w_e) = exp(w_e) G_e(d_k);
            # only the 32 DISTINCT tap columns are computed ----
            expt = pool.tile([128, KT], bf16, tag="expt")
            nc.scalar.activation(
                expt[:], d2[:], mybir.ActivationFunctionType.Exp,
                bias=wT, scale=invs[:],
            )

            # ---- ghat_col[p] = (c/Z) sum_e expt_w[e, p%32] / sigma_e via 4
            # concurrent col-tiled M=32 matmuls (same lhsT, 4 col strips) ----
            ghat_p = psum.tile([128, 1], f32, tag="ghat_p")
            for q in range(NQ):
                nc.tensor.matmul(
                    ghat_p[32 * q : 32 * q + 32, :], expt[:], rsb[:],
                    start=True, stop=True,
                    tile_position=(0, 32 * q),
                )
            ghat = pool.tile([128, 1], bf16, tag="ghat")
            nc.vector.tensor_scalar(
                ghat[:], ghat_p[:], rz[:], INV_SQRT_2PI,
                mybir.AluOpType.mult, mybir.AluOpType.mult,
            )

            # ---- conv: 4 concurrent 32x32-tile matmuls, quarter q ->
            # psum partition 32q: out[1, 256] = ghat[32q:+32]^T @ mT[32q:+32, :]
            op = psum.tile([128, QW], f32, tag="op")
            for q in range(NQ):
                nc.tensor.matmul(
                    op[32 * q : 32 * q + 1, :],
                    ghat[32 * q : 32 * q + 32, :],
                    mT[32 * q : 32 * q + 32, :],
                    start=True, stop=True,
                    tile_position=(32 * q, 32 * q),
                )

            # ---- fused PSUM evacuation + noise add; sparse-partition store
            outS = pool.tile([128, QW], f32, tag="outS")
            nc.vector.tensor_add(outS[:], op[:], nw[:])
            outS_ap = bass.AP(outS[:].tensor, 0, [[32 * QW, NQ], [1, QW]])
            nc.sync.dma_start(out_d[:], outS_ap)

    nc.compile()
    return nc


def kernel(X, sigma, weight, noise):
    global _compiled
    from concourse.bass_utils import run_bass_kernel_spmd

    X = np.ascontiguousarray(np.asarray(X, dtype=np.float32)).reshape(1, T)
    sigma = np.ascontiguousarray(np.asarray(sigma, dtype=np.float32)).reshape(E)
    weight = np.ascontiguousarray(np.asarray(weight, dtype=np.float32)).reshape(1, E)
    noise = np.ascontiguousarray(np.asarray(noise, dtype=np.float32)).reshape(1, T)

    if _compiled is None:
        _compiled = _build()
    nc = _compiled

    Xp = np.zeros(PADL + T + PADR, dtype=np.float32)
    Xp[PADL : PADL + T] = X[0]
    # windows[s] = Xp[s : s+QW]; row p of core c starts at 1024c + 256(p//32) + (p%32)
    win = np.lib.stride_tricks.sliding_window_view(Xp, QW)
    p = np.arange(128)
    row_off = QW * (p // KT) + (p % KT)

    in_maps = []
    for c in range(N_CORES):
        xsfull = np.empty((128, QW + 2), dtype=np.float32)
        xsfull[:, 0:QW] = win[c * TJ + row_off]
        xsfull[:, QW] = weight[0]
        xsfull[:, QW + 1] = sigma
        nw = np.ascontiguousarray(
            noise[0, c * TJ : (c + 1) * TJ].reshape(NQ, QW)
        )
        in_maps.append({
            "xstacka": np.ascontiguousarray(xsfull[0:64]),
            "xstackb": np.ascontiguousarray(xsfull[64:128]),
            "noisew": nw,
        })

    res = run_bass_kernel_spmd(nc, in_maps, core_ids=list(range(N_CORES)))
    out = np.empty((1, T), dtype=np.float32)
    for c in range(N_CORES):
        out[0, c * TJ : (c + 1) * TJ] = res.results[c]["out"].reshape(-1)
    return out
